# revision 9
# baseline (speedup 1.0000x reference)
"""DGCNN point-cloud classifier on 8 Trainium2 NeuronCores.

Sharding: data-parallel over the batch dim B=8 — one point cloud per core.
Each core runs 4 dynamic-kNN edge-conv layers + the 512->1024 linear +
global max/mean pooling locally; the pooled [2048] vectors are AllGathered
and every core computes the (tiny) batch-norm MLP head redundantly.

Edge-conv algebra: h[p,k] = [x_p, x_j - x_p] @ W + b with max over k
  = (x_p @ (Wt - Wb) + b) + max_k (x_j @ Wb)
so only per-point features ever go through matmuls; the kNN gather moves
F-dim rows of c = x @ Wb, done with gpsimd ap_gather in a feature-major
layout. Exact fp32 top-20 per row via DVE max8/match_replace/max_index.
"""
import numpy as np

N_CORES = 8
B, P, K, OUT = 8, 1024, 20, 40
T = P // 128  # 8 partition tiles per cloud
EPS = 1e-5
NEG = -1e30

# per-layer (C_in, F_out)
LAYERS = [(3, 64), (64, 64), (64, 128), (128, 256)]

_cache = {}


def _build():
    import concourse.bacc as bacc
    import concourse.mybir as mybir
    from concourse.tile import TileContext

    f32 = mybir.dt.float32
    u16 = mybir.dt.uint16
    i16 = mybir.dt.int16

    nc = bacc.Bacc(None, num_devices=N_CORES)

    # ---------------- I/O ----------------
    posT = nc.dram_tensor("posT", [3, P], f32, kind="ExternalInput")
    wsub, whalf, bvec = [], [], []
    for li, (C, F) in enumerate(LAYERS):
        wsub.append(nc.dram_tensor(f"wsub{li}", [C, F], f32, kind="ExternalInput"))
        whalf.append(nc.dram_tensor(f"whalf{li}", [C, F], f32, kind="ExternalInput"))
        bvec.append(nc.dram_tensor(f"bvec{li}", [F, 1], f32, kind="ExternalInput"))
    wm = nc.dram_tensor("wm", [512, 1024], f32, kind="ExternalInput")
    bm = nc.dram_tensor("bm", [1024, 1], f32, kind="ExternalInput")
    wa = nc.dram_tensor("wa", [2048, 512], f32, kind="ExternalInput")
    ba = nc.dram_tensor("ba", [512, 1], f32, kind="ExternalInput")
    ga = nc.dram_tensor("ga", [512, 1], f32, kind="ExternalInput")
    bea = nc.dram_tensor("bea", [512, 1], f32, kind="ExternalInput")
    wbh = nc.dram_tensor("wbh", [512, 256], f32, kind="ExternalInput")
    bbh = nc.dram_tensor("bbh", [256, 1], f32, kind="ExternalInput")
    gb = nc.dram_tensor("gb", [256, 1], f32, kind="ExternalInput")
    beb = nc.dram_tensor("beb", [256, 1], f32, kind="ExternalInput")
    wc = nc.dram_tensor("wc", [256, 40], f32, kind="ExternalInput")
    bc = nc.dram_tensor("bc", [40, 1], f32, kind="ExternalInput")
    y_out = nc.dram_tensor("y", [B, OUT], f32, kind="ExternalOutput")

    cc_in = nc.dram_tensor("cc_in", [1, 2048], f32, kind="Internal")
    cc_out = nc.dram_tensor("cc_out", [B, 2048], f32, kind="Internal",
                            addr_space="Shared")

    AG = mybir.AxisListType
    ALU = mybir.AluOpType
    ACTF = mybir.ActivationFunctionType

    with TileContext(nc) as tc:
        with tc.tile_pool(name="const", bufs=1) as cpool, \
             tc.tile_pool(name="dram", bufs=2, space="DRAM") as dpool:
            # ---------------- resident SBUF tensors ----------------
            ONES = cpool.tile([1, P], f32)
            nc.vector.memset(ONES[:], 1.0)
            NEGCOL = cpool.tile([128, 1], f32)
            nc.vector.memset(NEGCOL[:], -1.0)
            EPSC = cpool.tile([128, 1], f32)
            nc.vector.memset(EPSC[:], EPS)

            # feature buffers (lhs side, rows 0:C = x^T, row C = ones for l<=2)
            L1 = cpool.tile([4, P], f32)
            R1 = cpool.tile([4, P], f32)
            L2 = cpool.tile([65, P], f32)
            R2 = cpool.tile([65, P], f32)
            L3 = cpool.tile([65, P], f32)
            R3 = cpool.tile([65, P], f32)
            L4 = cpool.tile([128, P], f32)
            R4 = cpool.tile([128, P], f32)
            NEGSQ4 = cpool.tile([1, P], f32)
            X4a = cpool.tile([128, P], f32)
            X4b = cpool.tile([128, P], f32)
            Lbufs = [L1, L2, L3, L4]
            Rbufs = [R1, R2, R3, R4]

            AT1 = cpool.tile([128, P], f32)
            AT2 = cpool.tile([128, P], f32)
            CT1 = cpool.tile([128, P], f32)
            CT2 = cpool.tile([128, P], f32)

            # weights in SBUF
            ws_sb, wh_sb, b_sb = [], [], []
            for li, (C, F) in enumerate(LAYERS):
                w1 = cpool.tile([C, F], f32, tag=f"ws{li}")
                w2 = cpool.tile([C, F], f32, tag=f"wh{li}")
                bb_ = cpool.tile([min(F, 128), (F + 127) // 128], f32, tag=f"bv{li}")
                nc.sync.dma_start(w1[:], wsub[li][:])
                nc.sync.dma_start(w2[:], whalf[li][:])
                # bias [F,1] -> [128, F//128] col-blocks
                for mt in range((F + 127) // 128):
                    r0, r1 = 128 * mt, min(F, 128 * (mt + 1))
                    nc.sync.dma_start(bb_[0:r1 - r0, mt:mt + 1], bvec[li][r0:r1, :])
                ws_sb.append(w1)
                wh_sb.append(w2)
                b_sb.append(bb_)

            # Wm K-tiles: [64,64,128,128,128] rows
            wm_rows = [(0, 64), (64, 128), (128, 256), (256, 384), (384, 512)]
            wm_sb = []
            for i, (r0, r1) in enumerate(wm_rows):
                t_ = cpool.tile([r1 - r0, 1024], f32, tag=f"wm{i}")
                nc.sync.dma_start(t_[:], wm[r0:r1, :])
                wm_sb.append(t_)
            bm_sb = cpool.tile([128, 8], f32)
            for mt in range(8):
                nc.sync.dma_start(bm_sb[:, mt:mt + 1], bm[128 * mt:128 * (mt + 1), :])

            wa_sb = []
            for k in range(16):
                t_ = cpool.tile([128, 512], f32, tag=f"wa{k}")
                nc.sync.dma_start(t_[:], wa[128 * k:128 * (k + 1), :])
                wa_sb.append(t_)
            wbh_sb = []
            for k in range(4):
                t_ = cpool.tile([128, 256], f32, tag=f"wbh{k}")
                nc.sync.dma_start(t_[:], wbh[128 * k:128 * (k + 1), :])
                wbh_sb.append(t_)
            wc_sb = []
            for k in range(2):
                t_ = cpool.tile([128, 40], f32, tag=f"wc{k}")
                nc.sync.dma_start(t_[:], wc[128 * k:128 * (k + 1), :])
                wc_sb.append(t_)

            def colvec(dram, parts, blocks, tag):
                t_ = cpool.tile([parts, blocks], f32, tag=tag)
                for mt in range(blocks):
                    nc.sync.dma_start(t_[:, mt:mt + 1], dram[parts * mt:parts * (mt + 1), :])
                return t_

            ba_sb = colvec(ba, 128, 4, "ba")
            ga_sb = colvec(ga, 128, 4, "ga")
            bea_sb = colvec(bea, 128, 4, "bea")
            bbh_sb = colvec(bbh, 128, 2, "bbh")
            gb_sb = colvec(gb, 128, 2, "gb")
            beb_sb = colvec(beb, 128, 2, "beb")
            bc_sb = cpool.tile([40, 1], f32)
            nc.sync.dma_start(bc_sb[:], bc[:])

            # pos^T into L1 rows 0:3, ones rows via DMA from ONES
            nc.sync.dma_start(L1[0:3, :], posT[:])
            nc.sync.dma_start(L1[3:4, :], ONES[:])
            nc.sync.dma_start(L2[64:65, :], ONES[:])
            nc.sync.dma_start(L3[64:65, :], ONES[:])

            with tc.tile_pool(name="ps", bufs=2, space="PSUM") as pspool, \
                 tc.tile_pool(name="ps2", bufs=2, space="PSUM") as ps2pool, \
                 tc.tile_pool(name="work", bufs=3) as wpool, \
                 tc.tile_pool(name="gathp", bufs=2) as gpool, \
                 tc.tile_pool(name="idxp", bufs=3) as ipool:

                for li, (C, F) in enumerate(LAYERS):
                    Lb, Rb = Lbufs[li], Rbufs[li]
                    # R rows 0:C = 2*x^T
                    nc.scalar.activation(Rb[0:C, :], Lb[0:C, :], ACTF.Copy, scale=2.0)
                    # sqx = x^2, negsq = -(ones @ sqx)
                    sqx = wpool.tile([128, P], f32, tag="sqx")
                    nc.scalar.activation(sqx[0:C, :], Lb[0:C, :], ACTF.Square)
                    nps = ps2pool.tile([128, P], f32, tag="pre")
                    for n in range(2):
                        nc.tensor.matmul(nps[0:1, 512 * n:512 * (n + 1)],
                                         NEGCOL[0:C, :], sqx[0:C, 512 * n:512 * (n + 1)],
                                         start=True, stop=True)
                    if li == 3:
                        nc.scalar.activation(NEGSQ4[:], nps[0:1, :], ACTF.Copy)
                    else:
                        negsq = wpool.tile([1, P], f32, tag="negsq")
                        nc.scalar.activation(negsq[:], nps[0:1, :], ACTF.Copy)
                        nc.sync.dma_start(Rb[C:C + 1, :], negsq[:])

                    # a^T = wsub^T x + b ;  c^T = whalf^T x   (feature-major)
                    n_mt = (F + 127) // 128
                    ATs = [AT1, AT2][:n_mt]
                    CTs = [CT1, CT2][:n_mt]
                    for mt in range(n_mt):
                        Fm = min(128, F - 128 * mt)
                        aps = ps2pool.tile([128, P], f32, tag="pre")
                        cps = ps2pool.tile([128, P], f32, tag="pre")
                        for n in range(2):
                            nc.tensor.matmul(
                                aps[0:Fm, 512 * n:512 * (n + 1)],
                                ws_sb[li][:, 128 * mt:128 * mt + Fm],
                                Lb[0:C, 512 * n:512 * (n + 1)], start=True, stop=True)
                            nc.tensor.matmul(
                                cps[0:Fm, 512 * n:512 * (n + 1)],
                                wh_sb[li][:, 128 * mt:128 * mt + Fm],
                                Lb[0:C, 512 * n:512 * (n + 1)], start=True, stop=True)
                        nc.scalar.activation(ATs[mt][0:Fm, :], aps[0:Fm, :], ACTF.Identity,
                                             bias=b_sb[li][0:Fm, mt:mt + 1])
                        nc.scalar.activation(CTs[mt][0:Fm, :], cps[0:Fm, :], ACTF.Copy)

                    # destination of this layer's output features
                    if li < 3:
                        Lnext = Lbufs[li + 1]
                        outs_mt = [Lnext]
                    else:
                        outs_mt = [X4a, X4b]

                    # lhs/rhs K-tiles for the gram matmul
                    if li < 3:
                        lhs_kts = [(Lb, C + 1)]
                        rhs_kts = [(Rb, C + 1)]
                    else:
                        lhs_kts = [(Lb, 128), (ONES, 1)]
                        rhs_kts = [(Rb, 128), (NEGSQ4, 1)]

                    for t in range(T):
                        tc0, tc1 = 128 * t, 128 * (t + 1)
                        sps = pspool.tile([128, P], f32, tag="s")
                        for n in range(2):
                            for kt, ((lb, kk), (rb, _)) in enumerate(zip(lhs_kts, rhs_kts)):
                                nc.tensor.matmul(
                                    sps[:, 512 * n:512 * (n + 1)],
                                    lb[0:kk, tc0:tc1],
                                    rb[0:kk, 512 * n:512 * (n + 1)],
                                    start=(kt == 0), stop=(kt == len(lhs_kts) - 1))

                        # exact fp32 top-20 (values + indices) per row
                        vv = ipool.tile([128, 24], f32, tag="vv")
                        idxc = ipool.tile([128, 24], u16, tag="idxc")
                        scr = wpool.tile([128, P], f32, tag="scr")
                        nc.vector.max(vv[:, 0:8], sps[:])
                        nc.vector.max_index(idxc[:, 0:8], vv[:, 0:8], sps[:])
                        nc.vector.match_replace(scr[:], vv[:, 0:8], sps[:], NEG)
                        nc.vector.max(vv[:, 8:16], scr[:])
                        nc.vector.max_index(idxc[:, 8:16], vv[:, 8:16], scr[:])
                        nc.vector.match_replace(scr[:], vv[:, 8:16], scr[:], NEG)
                        nc.vector.max(vv[:, 16:24], scr[:])
                        nc.vector.max_index(idxc[:, 16:24], vv[:, 16:24], scr[:])

                        # idx list [128,20] -> wrapped [16,160] in DRAM, replicated
                        # into all 8 gpsimd core groups
                        idxd = dpool.tile([160, 16], i16, tag="idxd")
                        nc.sync.dma_start(idxd[:].bitcast(u16), idxc[:, 0:20])
                        idxw = ipool.tile([128, 160], i16, tag="idxw")
                        for cg in range(8):
                            nc.sync.dma_start(
                                idxw[16 * cg:16 * (cg + 1), :],
                                idxd[:].rearrange("j w -> w j"))

                        # gather c rows + k-reduce + add a^T
                        for mt in range(n_mt):
                            Fm = min(128, F - 128 * mt)
                            Fg = ((Fm + 15) // 16) * 16
                            gath = gpool.tile([128, K * 128], f32, tag="gath")
                            nc.gpsimd.ap_gather(
                                gath[0:Fg, :],
                                CTs[mt][0:Fg, :].rearrange("c (n d) -> c n d", d=1),
                                idxw[0:Fg, :],
                                channels=Fg, num_elems=P, d=1, num_idxs=K * 128)
                            # k=20 max tree on gpsimd: 16+4 -> 8 -> 4 -> 2 -> 1
                            g3 = gath[0:Fm, :].rearrange("c (p k) -> c p k", k=K)
                            r8 = wpool.tile([128, 128, 8], f32, tag="red8")
                            nc.gpsimd.tensor_tensor(r8[0:Fm, :, :], g3[:, :, 0:8],
                                                    g3[:, :, 8:16], op=ALU.max)
                            nc.gpsimd.tensor_tensor(r8[0:Fm, :, 0:4], r8[0:Fm, :, 0:4],
                                                    r8[0:Fm, :, 4:8], op=ALU.max)
                            nc.gpsimd.tensor_tensor(r8[0:Fm, :, 0:4], r8[0:Fm, :, 0:4],
                                                    g3[:, :, 16:20], op=ALU.max)
                            nc.gpsimd.tensor_tensor(r8[0:Fm, :, 0:2], r8[0:Fm, :, 0:2],
                                                    r8[0:Fm, :, 2:4], op=ALU.max)
                            nc.gpsimd.tensor_tensor(r8[0:Fm, :, 0:1], r8[0:Fm, :, 0:1],
                                                    r8[0:Fm, :, 1:2], op=ALU.max)
                            dst = outs_mt[mt] if li == 3 else outs_mt[0]
                            nc.gpsimd.tensor_add(dst[0:Fm, tc0:tc1],
                                                 r8[0:Fm, :, 0],
                                                 ATs[mt][0:Fm, tc0:tc1])

                # ---------------- cat @ Wm + pooling ----------------
                GP = cpool.tile([128, 16], f32)
                cat_kts = [(L2, 0, 64), (L3, 0, 64), (L4, 0, 128),
                           (X4a, 0, 128), (X4b, 0, 128)]
                for mt in range(8):
                    mc0, mc1 = 128 * mt, 128 * (mt + 1)
                    ops = pspool.tile([128, P], f32, tag="s")
                    for n in range(2):
                        for kt, ((buf, r0, r1), wmk) in enumerate(zip(cat_kts, wm_sb)):
                            nc.tensor.matmul(
                                ops[:, 512 * n:512 * (n + 1)],
                                wmk[:, mc0:mc1],
                                buf[r0:r1, 512 * n:512 * (n + 1)],
                                start=(kt == 0), stop=(kt == 4))
                    nc.vector.tensor_reduce(GP[:, mt:mt + 1], ops[:], axis=AG.X,
                                            op=ALU.max)
                    nc.vector.tensor_reduce(GP[:, 8 + mt:9 + mt], ops[:], axis=AG.X,
                                            op=ALU.add)
                # gmax += bm ; gmean = gmean/P + bm
                nc.vector.tensor_tensor(GP[:, 0:8], GP[:, 0:8], bm_sb[:], op=ALU.add)
                nc.vector.tensor_scalar(GP[:, 8:16], GP[:, 8:16], 1.0 / P, None,
                                        op0=ALU.mult)
                nc.vector.tensor_tensor(GP[:, 8:16], GP[:, 8:16], bm_sb[:], op=ALU.add)

                # pooled [2048] -> cc_in, AllGather
                for m in range(16):
                    nc.sync.dma_start(cc_in[0:1, 128 * m:128 * (m + 1)],
                                      GP[:, m:m + 1])
                nc.gpsimd.collective_compute(
                    "AllGather", ALU.bypass,
                    replica_groups=[list(range(N_CORES))],
                    ins=[cc_in[:].opt()], outs=[cc_out[:].opt()])

                # ---------------- head (redundant on every core) ----------------
                HT = cpool.tile([128, 128], f32)   # H^T K-tiles: col block k = [128,8]
                for k in range(16):
                    nc.sync.dma_start(
                        HT[:, 8 * k:8 * (k + 1)],
                        cc_out[:, 128 * k:128 * (k + 1)].rearrange("b f -> f b"))

                def bn_leaky(src, blocks, gamma, beta):
                    # src [128, 8*blocks]; batch-norm over free dim (batch) then leaky
                    for mt in range(blocks):
                        blk = src[:, 8 * mt:8 * (mt + 1)]
                        mu = wpool.tile([128, 1], f32, tag="mu")
                        nc.vector.tensor_reduce(mu[:], blk, axis=AG.X, op=ALU.add)
                        nc.vector.tensor_scalar(mu[:], mu[:], 1.0 / 8, None, op0=ALU.mult)
                        nc.vector.tensor_scalar(blk, blk, mu[:], None, op0=ALU.subtract)
                        sq2 = wpool.tile([128, 8], f32, tag="sq2")
                        nc.scalar.activation(sq2[:], blk, ACTF.Square)
                        var = wpool.tile([128, 1], f32, tag="var")
                        nc.vector.tensor_reduce(var[:], sq2[:], axis=AG.X, op=ALU.add)
                        nc.scalar.activation(var[:], var[:], ACTF.Sqrt,
                                             scale=1.0 / 8, bias=EPSC[:])
                        nc.vector.reciprocal(var[:], var[:])
                        nc.vector.tensor_scalar(blk, blk, var[:], None, op0=ALU.mult)
                        nc.vector.tensor_scalar(blk, blk, gamma[:, mt:mt + 1],
                                                beta[:, mt:mt + 1], op0=ALU.mult,
                                                op1=ALU.add)
                        lk = wpool.tile([128, 8], f32, tag="lk")
                        nc.vector.tensor_scalar(lk[:], blk, 0.2, None, op0=ALU.mult)
                        nc.vector.tensor_tensor(blk, blk, lk[:], op=ALU.max)

                HA = cpool.tile([128, 32], f32)
                for mt in range(4):
                    hps = ps2pool.tile([128, 8], f32, tag="pre")
                    for k in range(16):
                        nc.tensor.matmul(hps[:], wa_sb[k][:, 128 * mt:128 * (mt + 1)],
                                         HT[:, 8 * k:8 * (k + 1)],
                                         start=(k == 0), stop=(k == 15))
                    nc.scalar.activation(HA[:, 8 * mt:8 * (mt + 1)], hps[:], ACTF.Identity,
                                         bias=ba_sb[:, mt:mt + 1])
                bn_leaky(HA, 4, ga_sb, bea_sb)

                HB = cpool.tile([128, 16], f32)
                for mt in range(2):
                    hps = ps2pool.tile([128, 8], f32, tag="pre")
                    for k in range(4):
                        nc.tensor.matmul(hps[:], wbh_sb[k][:, 128 * mt:128 * (mt + 1)],
                                         HA[:, 8 * k:8 * (k + 1)],
                                         start=(k == 0), stop=(k == 3))
                    nc.scalar.activation(HB[:, 8 * mt:8 * (mt + 1)], hps[:], ACTF.Identity,
                                         bias=bbh_sb[:, mt:mt + 1])
                bn_leaky(HB, 2, gb_sb, beb_sb)

                ops2 = ps2pool.tile([128, 8], f32, tag="pre")
                for k in range(2):
                    nc.tensor.matmul(ops2[0:40, :], wc_sb[k][:, :],
                                     HB[:, 8 * k:8 * (k + 1)],
                                     start=(k == 0), stop=(k == 1))
                outs = cpool.tile([40, 8], f32)
                nc.scalar.activation(outs[:], ops2[0:40, :], ACTF.Identity, bias=bc_sb[:])
                nc.sync.dma_start(y_out[:].rearrange("b f -> f b"), outs[:])

    nc.finalize()
    return nc


def _prep_inputs(inputs):
    """Host-side sharding + weight reparametrization; all fp32."""
    f = np.float32
    pos = np.ascontiguousarray(inputs["pos"], dtype=f).reshape(B, P, 3)
    maps = []
    names = [("W1", "b1"), ("W2", "b2"), ("W3", "b3"), ("W4", "b4")]
    common = {}
    for li, (C, F) in enumerate(LAYERS):
        W = np.asarray(inputs[names[li][0]], dtype=f)
        b = np.asarray(inputs[names[li][1]], dtype=f)
        common[f"wsub{li}"] = np.ascontiguousarray(W[:C] - W[C:])
        common[f"whalf{li}"] = np.ascontiguousarray(W[C:])
        common[f"bvec{li}"] = b.reshape(F, 1)
    common["wm"] = np.asarray(inputs["Wm"], dtype=f)
    common["bm"] = np.asarray(inputs["bm"], dtype=f).reshape(1024, 1)
    common["wa"] = np.asarray(inputs["Wa"], dtype=f)
    common["ba"] = np.asarray(inputs["ba"], dtype=f).reshape(512, 1)
    common["ga"] = np.asarray(inputs["ga"], dtype=f).reshape(512, 1)
    common["bea"] = np.asarray(inputs["bea"], dtype=f).reshape(512, 1)
    common["wbh"] = np.asarray(inputs["Wb"], dtype=f)
    common["bbh"] = np.asarray(inputs["bb"], dtype=f).reshape(256, 1)
    common["gb"] = np.asarray(inputs["gb"], dtype=f).reshape(256, 1)
    common["beb"] = np.asarray(inputs["beb"], dtype=f).reshape(256, 1)
    common["wc"] = np.asarray(inputs["Wc"], dtype=f)
    common["bc"] = np.asarray(inputs["bc"], dtype=f).reshape(40, 1)
    for c in range(N_CORES):
        m = dict(common)
        m["posT"] = np.ascontiguousarray(pos[c].T)
        maps.append(m)
    return maps


def kernel(**inputs) -> np.ndarray:
    from concourse.bass_utils import run_bass_kernel_spmd

    if "nc" not in _cache:
        _cache["nc"] = _build()
    nc = _cache["nc"]
    in_maps = _prep_inputs(inputs)
    res = run_bass_kernel_spmd(nc, in_maps, core_ids=list(range(N_CORES)))
    return np.asarray(res.results[0]["y"], dtype=np.float32)


# revision 21
# speedup vs baseline: 1756.6100x; 1756.6100x over previous
"""DGCNN point-cloud classifier on 8 Trainium2 NeuronCores.

Sharding: data-parallel over the batch dim B=8 — one point cloud per core.
Each core runs 4 dynamic-kNN edge-conv layers + the 512->1024 linear +
global max/mean pooling locally; the pooled [2048] vectors are AllGathered
and every core computes the (tiny) batch-norm MLP head redundantly.

Edge-conv algebra: h[p,k] = [x_p, x_j - x_p] @ W + b with max over k
  = (x_p @ (Wt - Wb) + b) + max_k (x_j @ Wb)
so only per-point features ever go through matmuls; the kNN gather moves
F-dim rows of c = x @ Wb, done with gpsimd ap_gather in a feature-major
layout. Exact fp32 top-20 per row via DVE max8/match_replace/max_index.
"""
import numpy as np

N_CORES = 8
B, P, K, OUT = 8, 1024, 20, 40
T = P // 128  # 8 partition tiles per cloud
EPS = 1e-5
NEG = -1e30

# per-layer (C_in, F_out)
LAYERS = [(3, 64), (64, 64), (64, 128), (128, 256)]

_cache = {}


def _build():
    import concourse.bacc as bacc
    import concourse.mybir as mybir
    from concourse.tile import TileContext

    f32 = mybir.dt.float32
    u16 = mybir.dt.uint16
    i16 = mybir.dt.int16

    nc = bacc.Bacc(None, num_devices=N_CORES)

    # ---------------- I/O ----------------
    posT = nc.dram_tensor("posT", [3, P], f32, kind="ExternalInput")
    wsub, whalf, bvec = [], [], []
    for li, (C, F) in enumerate(LAYERS):
        wsub.append(nc.dram_tensor(f"wsub{li}", [C, F], f32, kind="ExternalInput"))
        whalf.append(nc.dram_tensor(f"whalf{li}", [C, F], f32, kind="ExternalInput"))
        bvec.append(nc.dram_tensor(f"bvec{li}", [F, 1], f32, kind="ExternalInput"))
    wm = nc.dram_tensor("wm", [512, 1024], f32, kind="ExternalInput")
    bm = nc.dram_tensor("bm", [1024, 1], f32, kind="ExternalInput")
    wa = nc.dram_tensor("wa", [2048, 512], f32, kind="ExternalInput")
    ba = nc.dram_tensor("ba", [512, 1], f32, kind="ExternalInput")
    ga = nc.dram_tensor("ga", [512, 1], f32, kind="ExternalInput")
    bea = nc.dram_tensor("bea", [512, 1], f32, kind="ExternalInput")
    wbh = nc.dram_tensor("wbh", [512, 256], f32, kind="ExternalInput")
    bbh = nc.dram_tensor("bbh", [256, 1], f32, kind="ExternalInput")
    gb = nc.dram_tensor("gb", [256, 1], f32, kind="ExternalInput")
    beb = nc.dram_tensor("beb", [256, 1], f32, kind="ExternalInput")
    wc = nc.dram_tensor("wc", [256, 40], f32, kind="ExternalInput")
    bc = nc.dram_tensor("bc", [40, 1], f32, kind="ExternalInput")
    y_out = nc.dram_tensor("y", [B, OUT], f32, kind="ExternalOutput")

    cc_in = nc.dram_tensor("cc_in", [1, 2048], f32, kind="Internal")
    cc_out = nc.dram_tensor("cc_out", [B, 2048], f32, kind="Internal",
                            addr_space="Shared")

    AG = mybir.AxisListType
    ALU = mybir.AluOpType
    ACTF = mybir.ActivationFunctionType

    with TileContext(nc) as tc:
        with tc.tile_pool(name="const", bufs=1) as cpool, \
             tc.tile_pool(name="dram", bufs=2, space="DRAM") as dpool:
            # ---------------- resident SBUF tensors ----------------
            ONES = cpool.tile([1, P], f32)
            nc.vector.memset(ONES[:], 1.0)
            NEGCOL = cpool.tile([128, 1], f32)
            nc.vector.memset(NEGCOL[:], -1.0)
            EPSC = cpool.tile([128, 1], f32)
            nc.vector.memset(EPSC[:], EPS)

            # feature buffers (lhs side, rows 0:C = x^T, row C = ones for l<=2)
            L1 = cpool.tile([4, P], f32)
            R1 = cpool.tile([4, P], f32)
            L2 = cpool.tile([65, P], f32)
            R2 = cpool.tile([65, P], f32)
            L3 = cpool.tile([65, P], f32)
            R3 = cpool.tile([65, P], f32)
            L4 = cpool.tile([128, P], f32)
            R4 = cpool.tile([128, P], f32)
            NEGSQ4 = cpool.tile([1, P], f32)
            X4a = cpool.tile([128, P], f32)
            X4b = cpool.tile([128, P], f32)
            Lbufs = [L1, L2, L3, L4]
            Rbufs = [R1, R2, R3, R4]

            AT1 = cpool.tile([128, P], f32)
            AT2 = cpool.tile([128, P], f32)
            CT1 = cpool.tile([128, P], f32)
            CT2 = cpool.tile([128, P], f32)

            # weights in SBUF
            ws_sb, wh_sb, b_sb = [], [], []
            for li, (C, F) in enumerate(LAYERS):
                w1 = cpool.tile([C, F], f32, tag=f"ws{li}")
                w2 = cpool.tile([C, F], f32, tag=f"wh{li}")
                bb_ = cpool.tile([min(F, 128), (F + 127) // 128], f32, tag=f"bv{li}")
                nc.sync.dma_start(w1[:], wsub[li][:])
                nc.sync.dma_start(w2[:], whalf[li][:])
                # bias [F,1] -> [128, F//128] col-blocks
                for mt in range((F + 127) // 128):
                    r0, r1 = 128 * mt, min(F, 128 * (mt + 1))
                    nc.sync.dma_start(bb_[0:r1 - r0, mt:mt + 1], bvec[li][r0:r1, :])
                ws_sb.append(w1)
                wh_sb.append(w2)
                b_sb.append(bb_)

            # Wm K-tiles: [64,64,128,128,128] rows. Tiles allocated now,
            # DMAs deferred until after the edge-conv layers so the layer-1
            # pipeline isn't stuck behind 7MB of head weights.
            wm_rows = [(0, 64), (64, 128), (128, 256), (256, 384), (384, 512)]
            f32r = mybir.dt.float32r
            wm_sb = []
            for i, (r0, r1) in enumerate(wm_rows):
                t_ = cpool.tile([r1 - r0, 1024], f32r, tag=f"wm{i}")
                wm_sb.append(t_)
            bm_sb = cpool.tile([128, 8], f32)
            wa_sb, wbh_sb, wc_sb = [], [], []
            for k in range(16):
                wa_t = cpool.tile([128, 512], f32, tag=f"wa{k}")
                wa_sb.append(wa_t)
            for k in range(4):
                wbh_t = cpool.tile([128, 256], f32, tag=f"wbh{k}")
                wbh_sb.append(wbh_t)
            for k in range(2):
                wc_t = cpool.tile([128, 40], f32, tag=f"wc{k}")
                wc_sb.append(wc_t)
            ba_sb = cpool.tile([128, 4], f32, tag="ba")
            ga_sb = cpool.tile([128, 4], f32, tag="ga")
            bea_sb = cpool.tile([128, 4], f32, tag="bea")
            bbh_sb = cpool.tile([128, 2], f32, tag="bbh")
            gb_sb = cpool.tile([128, 2], f32, tag="gb")
            beb_sb = cpool.tile([128, 2], f32, tag="beb")
            bc_sb = cpool.tile([40, 1], f32)

            def load_big_weights():
                for t_, (r0, r1) in zip(wm_sb, wm_rows):
                    nc.sync.dma_start(t_[:], wm[r0:r1, :].bitcast(f32r))
                for mt in range(8):
                    nc.sync.dma_start(bm_sb[:, mt:mt + 1],
                                      bm[128 * mt:128 * (mt + 1), :])
                for k in range(16):
                    nc.sync.dma_start(wa_sb[k][:], wa[128 * k:128 * (k + 1), :])
                for k in range(4):
                    nc.sync.dma_start(wbh_sb[k][:], wbh[128 * k:128 * (k + 1), :])
                for k in range(2):
                    nc.sync.dma_start(wc_sb[k][:], wc[128 * k:128 * (k + 1), :])
                for t_, dram, blocks in ((ba_sb, ba, 4), (ga_sb, ga, 4),
                                         (bea_sb, bea, 4), (bbh_sb, bbh, 2),
                                         (gb_sb, gb, 2), (beb_sb, beb, 2)):
                    for mt in range(blocks):
                        nc.sync.dma_start(t_[:, mt:mt + 1],
                                          dram[128 * mt:128 * (mt + 1), :])
                nc.sync.dma_start(bc_sb[:], bc[:])

            # pos^T into L1 rows 0:3, ones rows via DMA from ONES
            nc.sync.dma_start(L1[0:3, :], posT[:])
            nc.sync.dma_start(L1[3:4, :], ONES[:])
            nc.sync.dma_start(L2[64:65, :], ONES[:])
            nc.sync.dma_start(L3[64:65, :], ONES[:])

            with tc.tile_pool(name="ps", bufs=2, space="PSUM") as pspool, \
                 tc.tile_pool(name="ps2", bufs=2, space="PSUM") as ps2pool, \
                 tc.tile_pool(name="work", bufs=3) as wpool, \
                 tc.tile_pool(name="one", bufs=1) as opool, \
                 tc.tile_pool(name="gathp", bufs=2) as gpool, \
                 tc.tile_pool(name="idxp", bufs=3) as ipool:

                for li, (C, F) in enumerate(LAYERS):
                    if li == 1:
                        load_big_weights()
                    Lb, Rb = Lbufs[li], Rbufs[li]
                    # R rows 0:C = 2*x^T
                    nc.scalar.activation(Rb[0:C, :], Lb[0:C, :], ACTF.Copy, scale=2.0)
                    # sqx = x^2, negsq = -(ones @ sqx)
                    sqx = opool.tile([128, P], f32, tag="sqx")
                    nc.scalar.activation(sqx[0:C, :], Lb[0:C, :], ACTF.Square)
                    nps = ps2pool.tile([128, P], f32, tag="pre")
                    for n in range(2):
                        nc.tensor.matmul(nps[0:1, 512 * n:512 * (n + 1)],
                                         NEGCOL[0:C, :], sqx[0:C, 512 * n:512 * (n + 1)],
                                         start=True, stop=True)
                    if li == 3:
                        nc.scalar.activation(NEGSQ4[:], nps[0:1, :], ACTF.Copy)
                    else:
                        negsq = wpool.tile([1, P], f32, tag="negsq")
                        nc.scalar.activation(negsq[:], nps[0:1, :], ACTF.Copy)
                        nc.sync.dma_start(Rb[C:C + 1, :], negsq[:])

                    # a^T = wsub^T x + b ;  c^T = whalf^T x   (feature-major)
                    n_mt = (F + 127) // 128
                    ATs = [AT1, AT2][:n_mt]
                    CTs = [CT1, CT2][:n_mt]
                    for mt in range(n_mt):
                        Fm = min(128, F - 128 * mt)
                        aps = ps2pool.tile([128, P], f32, tag="pre")
                        cps = ps2pool.tile([128, P], f32, tag="pre")
                        for n in range(2):
                            nc.tensor.matmul(
                                aps[0:Fm, 512 * n:512 * (n + 1)],
                                ws_sb[li][:, 128 * mt:128 * mt + Fm],
                                Lb[0:C, 512 * n:512 * (n + 1)], start=True, stop=True)
                            nc.tensor.matmul(
                                cps[0:Fm, 512 * n:512 * (n + 1)],
                                wh_sb[li][:, 128 * mt:128 * mt + Fm],
                                Lb[0:C, 512 * n:512 * (n + 1)], start=True, stop=True)
                        nc.scalar.activation(ATs[mt][0:Fm, :], aps[0:Fm, :], ACTF.Identity,
                                             bias=b_sb[li][0:Fm, mt:mt + 1])
                        nc.scalar.activation(CTs[mt][0:Fm, :], cps[0:Fm, :], ACTF.Copy)

                    # destination of this layer's output features
                    if li < 3:
                        Lnext = Lbufs[li + 1]
                        outs_mt = [Lnext]
                    else:
                        outs_mt = [X4a, X4b]

                    # lhs/rhs K-tiles for the gram matmul
                    if li < 3:
                        lhs_kts = [(Lb, C + 1)]
                        rhs_kts = [(Rb, C + 1)]
                    else:
                        lhs_kts = [(Lb, 128), (ONES, 1)]
                        rhs_kts = [(Rb, 128), (NEGSQ4, 1)]

                    pending = []

                    def flush_pending():
                        for (g_, Fm_, dst_, mt_, a0, a1) in pending:
                            red = wpool.tile([128, 128], f32, tag="red")
                            nc.vector.tensor_reduce(
                                red[0:Fm_, :],
                                g_[0:Fm_, :].rearrange("c (p k) -> c p k", k=K),
                                axis=AG.X, op=ALU.max)
                            nc.gpsimd.tensor_add(dst_[0:Fm_, a0:a1], red[0:Fm_, :],
                                                 ATs[mt_][0:Fm_, a0:a1])
                        pending.clear()

                    for t in range(T):
                        tc0, tc1 = 128 * t, 128 * (t + 1)
                        sps = pspool.tile([128, P], f32, tag="s")
                        for n in range(2):
                            for kt, ((lb, kk), (rb, _)) in enumerate(zip(lhs_kts, rhs_kts)):
                                nc.tensor.matmul(
                                    sps[:, 512 * n:512 * (n + 1)],
                                    lb[0:kk, tc0:tc1],
                                    rb[0:kk, 512 * n:512 * (n + 1)],
                                    start=(kt == 0), stop=(kt == len(lhs_kts) - 1))

                        # exact fp32 top-20 (values + indices) per row
                        vv = ipool.tile([128, 24], f32, tag="vv")
                        idxc = ipool.tile([128, 24], u16, tag="idxc")
                        scr = wpool.tile([128, P], f32, tag="scr")
                        nc.vector.max(vv[:, 0:8], sps[:])
                        nc.vector.max_index(idxc[:, 0:8], vv[:, 0:8], sps[:])
                        nc.vector.match_replace(scr[:], vv[:, 0:8], sps[:], NEG)
                        nc.vector.max(vv[:, 8:16], scr[:])
                        nc.vector.max_index(idxc[:, 8:16], vv[:, 8:16], scr[:])
                        nc.vector.match_replace(scr[:], vv[:, 8:16], scr[:], NEG)
                        nc.vector.max(vv[:, 16:24], scr[:])
                        nc.vector.max_index(idxc[:, 16:24], vv[:, 16:24], scr[:])

                        flush_pending()

                        # idx list [128,20] -> wrapped [16,160] in DRAM, replicated
                        # into all 8 gpsimd core groups
                        idxd = dpool.tile([160, 16], i16, tag="idxd")
                        nc.sync.dma_start(idxd[:].bitcast(u16), idxc[:, 0:20])
                        idxw = ipool.tile([128, 160], i16, tag="idxw")
                        idxv = idxd[:].rearrange("j w -> w j")
                        for cg in range(8):
                            nc.sync.dma_start(
                                idxw[16 * cg:16 * (cg + 1), :], idxv)

                        # gather c rows + k-reduce + add a^T
                        for mt in range(n_mt):
                            Fm = min(128, F - 128 * mt)
                            Fg = ((Fm + 15) // 16) * 16
                            gath = gpool.tile([128, K * 128], f32, tag="gath")
                            nc.gpsimd.ap_gather(
                                gath[0:Fg, :],
                                CTs[mt][0:Fg, :].rearrange("c (n d) -> c n d", d=1),
                                idxw[0:Fg, :],
                                channels=Fg, num_elems=P, d=1, num_idxs=K * 128)
                            dst = outs_mt[mt] if li == 3 else outs_mt[0]
                            pending.append((gath, Fm, dst, mt, tc0, tc1))

                    flush_pending()

                # ---------------- cat @ Wm + pooling ----------------
                GP = cpool.tile([128, 16], f32)
                cat_kts = [(L2, 0, 64), (L3, 0, 64), (L4, 0, 128),
                           (X4a, 0, 128), (X4b, 0, 128)]
                # round cat features into f32r tiles (ACT is idle; selection
                # is all done so rounding is safe here)
                catr = []
                for i, (buf, r0, r1) in enumerate(cat_kts):
                    cr_ = cpool.tile([r1 - r0, P], f32r, tag=f"catr{i}")
                    nc.scalar.activation(cr_[:], buf[r0:r1, :], ACTF.Copy)
                    catr.append((cr_, 0, r1 - r0))
                cat_kts = catr
                for mt in range(8):
                    mc0, mc1 = 128 * mt, 128 * (mt + 1)
                    ops = pspool.tile([128, P], f32, tag="s")
                    for n in range(2):
                        for kt, ((buf, r0, r1), wmk) in enumerate(zip(cat_kts, wm_sb)):
                            nc.tensor.matmul(
                                ops[:, 512 * n:512 * (n + 1)],
                                wmk[:, mc0:mc1],
                                buf[r0:r1, 512 * n:512 * (n + 1)],
                                start=(kt == 0), stop=(kt == 4))
                    nc.vector.tensor_reduce(GP[:, mt:mt + 1], ops[:], axis=AG.X,
                                            op=ALU.max)
                    # gmean: sum via ACT accumulate during PSUM->SBUF drain
                    osb = opool.tile([128, P], f32, tag="osb")
                    nc.scalar.activation(osb[:], ops[:], ACTF.Copy,
                                         accum_out=GP[:, 8 + mt:9 + mt])
                # gmax += bm ; gmean = gmean/P + bm
                nc.vector.tensor_tensor(GP[:, 0:8], GP[:, 0:8], bm_sb[:], op=ALU.add)
                nc.vector.tensor_scalar(GP[:, 8:16], GP[:, 8:16], 1.0 / P, None,
                                        op0=ALU.mult)
                nc.vector.tensor_tensor(GP[:, 8:16], GP[:, 8:16], bm_sb[:], op=ALU.add)

                # pooled [2048] -> cc_in, AllGather
                for m in range(16):
                    nc.sync.dma_start(cc_in[0:1, 128 * m:128 * (m + 1)],
                                      GP[:, m:m + 1])
                nc.gpsimd.collective_compute(
                    "AllGather", ALU.bypass,
                    replica_groups=[list(range(N_CORES))],
                    ins=[cc_in[:].opt()], outs=[cc_out[:].opt()])

                # ---------------- head (redundant on every core) ----------------
                HT = cpool.tile([128, 128], f32)   # H^T K-tiles: col block k = [128,8]
                for k in range(16):
                    nc.sync.dma_start(
                        HT[:, 8 * k:8 * (k + 1)],
                        cc_out[:, 128 * k:128 * (k + 1)].rearrange("b f -> f b"))

                def bn_leaky(src, blocks, gamma, beta):
                    # src [128, 8*blocks]; batch-norm over free dim (batch) then leaky
                    for mt in range(blocks):
                        blk = src[:, 8 * mt:8 * (mt + 1)]
                        mu = wpool.tile([128, 1], f32, tag="mu")
                        nc.vector.tensor_reduce(mu[:], blk, axis=AG.X, op=ALU.add)
                        nc.vector.tensor_scalar(mu[:], mu[:], 1.0 / 8, None, op0=ALU.mult)
                        nc.vector.tensor_scalar(blk, blk, mu[:], None, op0=ALU.subtract)
                        sq2 = wpool.tile([128, 8], f32, tag="sq2")
                        nc.scalar.activation(sq2[:], blk, ACTF.Square)
                        var = wpool.tile([128, 1], f32, tag="var")
                        nc.vector.tensor_reduce(var[:], sq2[:], axis=AG.X, op=ALU.add)
                        nc.scalar.activation(var[:], var[:], ACTF.Sqrt,
                                             scale=1.0 / 8, bias=EPSC[:])
                        nc.vector.reciprocal(var[:], var[:])
                        nc.vector.tensor_scalar(blk, blk, var[:], None, op0=ALU.mult)
                        nc.vector.tensor_scalar(blk, blk, gamma[:, mt:mt + 1],
                                                beta[:, mt:mt + 1], op0=ALU.mult,
                                                op1=ALU.add)
                        lk = wpool.tile([128, 8], f32, tag="lk")
                        nc.vector.tensor_scalar(lk[:], blk, 0.2, None, op0=ALU.mult)
                        nc.vector.tensor_tensor(blk, blk, lk[:], op=ALU.max)

                HA = cpool.tile([128, 32], f32)
                for mt in range(4):
                    hps = ps2pool.tile([128, 8], f32, tag="pre")
                    for k in range(16):
                        nc.tensor.matmul(hps[:], wa_sb[k][:, 128 * mt:128 * (mt + 1)],
                                         HT[:, 8 * k:8 * (k + 1)],
                                         start=(k == 0), stop=(k == 15))
                    nc.scalar.activation(HA[:, 8 * mt:8 * (mt + 1)], hps[:], ACTF.Identity,
                                         bias=ba_sb[:, mt:mt + 1])
                bn_leaky(HA, 4, ga_sb, bea_sb)

                HB = cpool.tile([128, 16], f32)
                for mt in range(2):
                    hps = ps2pool.tile([128, 8], f32, tag="pre")
                    for k in range(4):
                        nc.tensor.matmul(hps[:], wbh_sb[k][:, 128 * mt:128 * (mt + 1)],
                                         HA[:, 8 * k:8 * (k + 1)],
                                         start=(k == 0), stop=(k == 3))
                    nc.scalar.activation(HB[:, 8 * mt:8 * (mt + 1)], hps[:], ACTF.Identity,
                                         bias=bbh_sb[:, mt:mt + 1])
                bn_leaky(HB, 2, gb_sb, beb_sb)

                ops2 = ps2pool.tile([128, 8], f32, tag="pre")
                for k in range(2):
                    nc.tensor.matmul(ops2[0:40, :], wc_sb[k][:, :],
                                     HB[:, 8 * k:8 * (k + 1)],
                                     start=(k == 0), stop=(k == 1))
                outs = cpool.tile([40, 8], f32)
                nc.scalar.activation(outs[:], ops2[0:40, :], ACTF.Identity, bias=bc_sb[:])
                nc.sync.dma_start(y_out[:].rearrange("b f -> f b"), outs[:])

    nc.finalize()
    return nc


def _prep_inputs(inputs):
    """Host-side sharding + weight reparametrization; all fp32."""
    f = np.float32
    pos = np.ascontiguousarray(inputs["pos"], dtype=f).reshape(B, P, 3)
    maps = []
    names = [("W1", "b1"), ("W2", "b2"), ("W3", "b3"), ("W4", "b4")]
    common = {}
    for li, (C, F) in enumerate(LAYERS):
        W = np.asarray(inputs[names[li][0]], dtype=f)
        b = np.asarray(inputs[names[li][1]], dtype=f)
        common[f"wsub{li}"] = np.ascontiguousarray(W[:C] - W[C:])
        common[f"whalf{li}"] = np.ascontiguousarray(W[C:])
        common[f"bvec{li}"] = b.reshape(F, 1)
    common["wm"] = np.asarray(inputs["Wm"], dtype=f)
    common["bm"] = np.asarray(inputs["bm"], dtype=f).reshape(1024, 1)
    common["wa"] = np.asarray(inputs["Wa"], dtype=f)
    common["ba"] = np.asarray(inputs["ba"], dtype=f).reshape(512, 1)
    common["ga"] = np.asarray(inputs["ga"], dtype=f).reshape(512, 1)
    common["bea"] = np.asarray(inputs["bea"], dtype=f).reshape(512, 1)
    common["wbh"] = np.asarray(inputs["Wb"], dtype=f)
    common["bbh"] = np.asarray(inputs["bb"], dtype=f).reshape(256, 1)
    common["gb"] = np.asarray(inputs["gb"], dtype=f).reshape(256, 1)
    common["beb"] = np.asarray(inputs["beb"], dtype=f).reshape(256, 1)
    common["wc"] = np.asarray(inputs["Wc"], dtype=f)
    common["bc"] = np.asarray(inputs["bc"], dtype=f).reshape(40, 1)
    for c in range(N_CORES):
        m = dict(common)
        m["posT"] = np.ascontiguousarray(pos[c].T)
        maps.append(m)
    return maps


def kernel(**inputs) -> np.ndarray:
    from concourse.bass_utils import run_bass_kernel_spmd

    if "nc" not in _cache:
        _cache["nc"] = _build()
    nc = _cache["nc"]
    in_maps = _prep_inputs(inputs)
    res = run_bass_kernel_spmd(nc, in_maps, core_ids=list(range(N_CORES)))
    return np.asarray(res.results[0]["y"], dtype=np.float32)


# revision 25
# speedup vs baseline: 2072.1472x; 1.1796x over previous
"""DGCNN point-cloud classifier on 8 Trainium2 NeuronCores.

Sharding: data-parallel over the batch dim B=8 — one point cloud per core.
Each core runs 4 dynamic-kNN edge-conv layers + the 512->1024 linear +
global max/mean pooling locally; the pooled [2048] vectors are AllGathered
and every core computes the (tiny) batch-norm MLP head redundantly.

Edge-conv algebra: h[p,k] = [x_p, x_j - x_p] @ W + b with max over k
  = (x_p @ (Wt - Wb) + b) + max_k (x_j @ Wb)
so only per-point features ever go through matmuls; the kNN gather moves
F-dim rows of c = x @ Wb, done with gpsimd ap_gather in a feature-major
layout. Exact fp32 top-20 per row via DVE max8/match_replace/max_index.
"""
import numpy as np

N_CORES = 8
B, P, K, OUT = 8, 1024, 20, 40
T = P // 128  # 8 partition tiles per cloud
EPS = 1e-5
NEG = -1e30

# per-layer (C_in, F_out)
LAYERS = [(3, 64), (64, 64), (64, 128), (128, 256)]

_cache = {}


def _build():
    import concourse.bacc as bacc
    import concourse.mybir as mybir
    from concourse.tile import TileContext

    f32 = mybir.dt.float32
    u16 = mybir.dt.uint16
    i16 = mybir.dt.int16

    nc = bacc.Bacc(None, num_devices=N_CORES)

    # ---------------- I/O ----------------
    posT = nc.dram_tensor("posT", [3, P], f32, kind="ExternalInput")
    wsub, whalf, bvec = [], [], []
    for li, (C, F) in enumerate(LAYERS):
        wsub.append(nc.dram_tensor(f"wsub{li}", [C, F], f32, kind="ExternalInput"))
        whalf.append(nc.dram_tensor(f"whalf{li}", [C, F], f32, kind="ExternalInput"))
        bvec.append(nc.dram_tensor(f"bvec{li}", [F, 1], f32, kind="ExternalInput"))
    wm = nc.dram_tensor("wm", [512, 1024], f32, kind="ExternalInput")
    bm = nc.dram_tensor("bm", [1024, 1], f32, kind="ExternalInput")
    wa = nc.dram_tensor("wa", [2048, 512], f32, kind="ExternalInput")
    ba = nc.dram_tensor("ba", [512, 1], f32, kind="ExternalInput")
    ga = nc.dram_tensor("ga", [512, 1], f32, kind="ExternalInput")
    bea = nc.dram_tensor("bea", [512, 1], f32, kind="ExternalInput")
    wbh = nc.dram_tensor("wbh", [512, 256], f32, kind="ExternalInput")
    bbh = nc.dram_tensor("bbh", [256, 1], f32, kind="ExternalInput")
    gb = nc.dram_tensor("gb", [256, 1], f32, kind="ExternalInput")
    beb = nc.dram_tensor("beb", [256, 1], f32, kind="ExternalInput")
    wc = nc.dram_tensor("wc", [256, 40], f32, kind="ExternalInput")
    bc = nc.dram_tensor("bc", [40, 1], f32, kind="ExternalInput")
    y_out = nc.dram_tensor("y", [B, OUT], f32, kind="ExternalOutput")

    cc_in = nc.dram_tensor("cc_in", [1, 2048], f32, kind="Internal")
    cc_out = nc.dram_tensor("cc_out", [B, 2048], f32, kind="Internal",
                            addr_space="Shared")

    AG = mybir.AxisListType
    ALU = mybir.AluOpType
    ACTF = mybir.ActivationFunctionType

    with TileContext(nc) as tc:
        with tc.tile_pool(name="const", bufs=1) as cpool, \
             tc.tile_pool(name="dram", bufs=2, space="DRAM") as dpool:
            # ---------------- resident SBUF tensors ----------------
            ONES = cpool.tile([1, P], f32)
            nc.vector.memset(ONES[:], 1.0)
            NEGCOL = cpool.tile([128, 1], f32)
            nc.vector.memset(NEGCOL[:], -1.0)
            EPSC = cpool.tile([128, 1], f32)
            nc.vector.memset(EPSC[:], EPS)

            # feature buffers (lhs side, rows 0:C = x^T, row C = ones for l<=2)
            L1 = cpool.tile([4, P], f32)
            R1 = cpool.tile([4, P], f32)
            L2 = cpool.tile([65, P], f32)
            R2 = cpool.tile([65, P], f32)
            L3 = cpool.tile([65, P], f32)
            R3 = cpool.tile([65, P], f32)
            L4 = cpool.tile([128, P], f32)
            R4 = cpool.tile([128, P], f32)
            NEGSQ4 = cpool.tile([1, P], f32)
            X4a = cpool.tile([128, P], f32)
            X4b = cpool.tile([128, P], f32)
            Lbufs = [L1, L2, L3, L4]
            Rbufs = [R1, R2, R3, R4]

            AT1 = cpool.tile([128, P], f32)
            AT2 = cpool.tile([128, P], f32)
            CT1 = cpool.tile([128, P], f32)
            CT2 = cpool.tile([128, P], f32)

            # weights in SBUF
            ws_sb, wh_sb, b_sb = [], [], []
            for li, (C, F) in enumerate(LAYERS):
                w1 = cpool.tile([C, F], f32, tag=f"ws{li}")
                w2 = cpool.tile([C, F], f32, tag=f"wh{li}")
                bb_ = cpool.tile([min(F, 128), (F + 127) // 128], f32, tag=f"bv{li}")
                nc.sync.dma_start(w1[:], wsub[li][:])
                nc.sync.dma_start(w2[:], whalf[li][:])
                # bias [F,1] -> [128, F//128] col-blocks
                for mt in range((F + 127) // 128):
                    r0, r1 = 128 * mt, min(F, 128 * (mt + 1))
                    nc.sync.dma_start(bb_[0:r1 - r0, mt:mt + 1], bvec[li][r0:r1, :])
                ws_sb.append(w1)
                wh_sb.append(w2)
                b_sb.append(bb_)

            # Wm K-tiles: [64,64,128,128,128] rows. Tiles allocated now,
            # DMAs deferred until after the edge-conv layers so the layer-1
            # pipeline isn't stuck behind 7MB of head weights.
            wm_rows = [(0, 64), (64, 128), (128, 256), (256, 384), (384, 512)]
            f32r = mybir.dt.float32r
            wm_sb = []
            for i, (r0, r1) in enumerate(wm_rows):
                t_ = cpool.tile([r1 - r0, 1024], f32r, tag=f"wm{i}")
                wm_sb.append(t_)
            bm_sb = cpool.tile([128, 8], f32)
            wa_sb, wbh_sb, wc_sb = [], [], []
            for k in range(16):
                wa_t = cpool.tile([128, 512], f32, tag=f"wa{k}")
                wa_sb.append(wa_t)
            for k in range(4):
                wbh_t = cpool.tile([128, 256], f32, tag=f"wbh{k}")
                wbh_sb.append(wbh_t)
            for k in range(2):
                wc_t = cpool.tile([128, 40], f32, tag=f"wc{k}")
                wc_sb.append(wc_t)
            ba_sb = cpool.tile([128, 4], f32, tag="ba")
            ga_sb = cpool.tile([128, 4], f32, tag="ga")
            bea_sb = cpool.tile([128, 4], f32, tag="bea")
            bbh_sb = cpool.tile([128, 2], f32, tag="bbh")
            gb_sb = cpool.tile([128, 2], f32, tag="gb")
            beb_sb = cpool.tile([128, 2], f32, tag="beb")
            bc_sb = cpool.tile([40, 1], f32)

            def load_big_weights():
                for t_, (r0, r1) in zip(wm_sb, wm_rows):
                    nc.sync.dma_start(t_[:], wm[r0:r1, :].bitcast(f32r))
                for mt in range(8):
                    nc.sync.dma_start(bm_sb[:, mt:mt + 1],
                                      bm[128 * mt:128 * (mt + 1), :])
                for k in range(16):
                    nc.sync.dma_start(wa_sb[k][:], wa[128 * k:128 * (k + 1), :])
                for k in range(4):
                    nc.sync.dma_start(wbh_sb[k][:], wbh[128 * k:128 * (k + 1), :])
                for k in range(2):
                    nc.sync.dma_start(wc_sb[k][:], wc[128 * k:128 * (k + 1), :])
                for t_, dram, blocks in ((ba_sb, ba, 4), (ga_sb, ga, 4),
                                         (bea_sb, bea, 4), (bbh_sb, bbh, 2),
                                         (gb_sb, gb, 2), (beb_sb, beb, 2)):
                    for mt in range(blocks):
                        nc.sync.dma_start(t_[:, mt:mt + 1],
                                          dram[128 * mt:128 * (mt + 1), :])
                nc.sync.dma_start(bc_sb[:], bc[:])

            # pos^T into L1 rows 0:3, ones rows via DMA from ONES
            nc.sync.dma_start(L1[0:3, :], posT[:])
            nc.sync.dma_start(L1[3:4, :], ONES[:])
            nc.sync.dma_start(L2[64:65, :], ONES[:])
            nc.sync.dma_start(L3[64:65, :], ONES[:])

            with tc.tile_pool(name="ps", bufs=2, space="PSUM") as pspool, \
                 tc.tile_pool(name="ps2", bufs=2, space="PSUM") as ps2pool, \
                 tc.tile_pool(name="work", bufs=2) as wpool, \
                 tc.tile_pool(name="one", bufs=1) as opool, \
                 tc.tile_pool(name="gathp", bufs=3) as gpool, \
                 tc.tile_pool(name="idxp", bufs=3) as ipool:

                for li, (C, F) in enumerate(LAYERS):
                    if li == 1:
                        load_big_weights()
                    Lb, Rb = Lbufs[li], Rbufs[li]
                    # R rows 0:C = 2*x^T
                    nc.scalar.activation(Rb[0:C, :], Lb[0:C, :], ACTF.Copy, scale=2.0)
                    # sqx = x^2, negsq = -(ones @ sqx)
                    sqx = opool.tile([128, P], f32, tag="sqx")
                    nc.scalar.activation(sqx[0:C, :], Lb[0:C, :], ACTF.Square)
                    nps = ps2pool.tile([128, P], f32, tag="pre")
                    for n in range(2):
                        nc.tensor.matmul(nps[0:1, 512 * n:512 * (n + 1)],
                                         NEGCOL[0:C, :], sqx[0:C, 512 * n:512 * (n + 1)],
                                         start=True, stop=True)
                    if li == 3:
                        nc.scalar.activation(NEGSQ4[:], nps[0:1, :], ACTF.Copy)
                    else:
                        negsq = wpool.tile([1, P], f32, tag="negsq")
                        nc.scalar.activation(negsq[:], nps[0:1, :], ACTF.Copy)
                        nc.sync.dma_start(Rb[C:C + 1, :], negsq[:])

                    # a^T = wsub^T x + b ;  c^T = whalf^T x   (feature-major)
                    n_mt = (F + 127) // 128
                    ATs = [AT1, AT2][:n_mt]
                    CTs = [CT1, CT2][:n_mt]
                    for mt in range(n_mt):
                        Fm = min(128, F - 128 * mt)
                        aps = ps2pool.tile([128, P], f32, tag="pre")
                        cps = ps2pool.tile([128, P], f32, tag="pre")
                        for n in range(2):
                            nc.tensor.matmul(
                                aps[0:Fm, 512 * n:512 * (n + 1)],
                                ws_sb[li][:, 128 * mt:128 * mt + Fm],
                                Lb[0:C, 512 * n:512 * (n + 1)], start=True, stop=True)
                            nc.tensor.matmul(
                                cps[0:Fm, 512 * n:512 * (n + 1)],
                                wh_sb[li][:, 128 * mt:128 * mt + Fm],
                                Lb[0:C, 512 * n:512 * (n + 1)], start=True, stop=True)
                        nc.scalar.activation(ATs[mt][0:Fm, :], aps[0:Fm, :], ACTF.Identity,
                                             bias=b_sb[li][0:Fm, mt:mt + 1])
                        nc.scalar.activation(CTs[mt][0:Fm, :], cps[0:Fm, :], ACTF.Copy)

                    # destination of this layer's output features
                    if li < 3:
                        Lnext = Lbufs[li + 1]
                        outs_mt = [Lnext]
                    else:
                        outs_mt = [X4a, X4b]

                    # lhs/rhs K-tiles for the gram matmul
                    if li < 3:
                        lhs_kts = [(Lb, C + 1)]
                        rhs_kts = [(Rb, C + 1)]
                    else:
                        lhs_kts = [(Lb, 128), (ONES, 1)]
                        rhs_kts = [(Rb, 128), (NEGSQ4, 1)]

                    pending = []

                    def flush_pending():
                        # deprioritize so the scheduler orders these after the
                        # next tile's top-k instead of stalling DVE on the
                        # gather chain
                        with tc.high_priority(offset=-60):
                            for (g_, Fm_, dst_, mt_, a0, a1) in pending:
                                red = wpool.tile([128, 128], f32, tag="red")
                                nc.vector.tensor_reduce(
                                    red[0:Fm_, :],
                                    g_[0:Fm_, :].rearrange("c (p k) -> c p k", k=K),
                                    axis=AG.X, op=ALU.max)
                                nc.gpsimd.tensor_add(dst_[0:Fm_, a0:a1],
                                                     red[0:Fm_, :],
                                                     ATs[mt_][0:Fm_, a0:a1])
                        pending.clear()

                    for t in range(T):
                        tc0, tc1 = 128 * t, 128 * (t + 1)
                        sps = pspool.tile([128, P], f32, tag="s")
                        for n in range(2):
                            for kt, ((lb, kk), (rb, _)) in enumerate(zip(lhs_kts, rhs_kts)):
                                nc.tensor.matmul(
                                    sps[:, 512 * n:512 * (n + 1)],
                                    lb[0:kk, tc0:tc1],
                                    rb[0:kk, 512 * n:512 * (n + 1)],
                                    start=(kt == 0), stop=(kt == len(lhs_kts) - 1))

                        # exact fp32 top-20 (values + indices) per row
                        vv = ipool.tile([128, 24], f32, tag="vv")
                        idxc = ipool.tile([128, 24], u16, tag="idxc")
                        scr = wpool.tile([128, P], f32, tag="scr")
                        nc.vector.max(vv[:, 0:8], sps[:])
                        nc.vector.max_index(idxc[:, 0:8], vv[:, 0:8], sps[:])
                        nc.vector.match_replace(scr[:], vv[:, 0:8], sps[:], NEG)
                        nc.vector.max(vv[:, 8:16], scr[:])
                        nc.vector.max_index(idxc[:, 8:16], vv[:, 8:16], scr[:])
                        nc.vector.match_replace(scr[:], vv[:, 8:16], scr[:], NEG)
                        nc.vector.max(vv[:, 16:24], scr[:])
                        nc.vector.max_index(idxc[:, 16:24], vv[:, 16:24], scr[:])

                        flush_pending()

                        # idx list [128,20] -> wrapped [16,160] in DRAM, replicated
                        # into all 8 gpsimd core groups
                        idxd = dpool.tile([160, 16], i16, tag="idxd")
                        nc.sync.dma_start(idxd[:].bitcast(u16), idxc[:, 0:20])
                        idxw = ipool.tile([128, 160], i16, tag="idxw")
                        idxv = idxd[:].rearrange("j w -> w j")
                        for cg in range(8):
                            nc.sync.dma_start(
                                idxw[16 * cg:16 * (cg + 1), :], idxv)

                        # gather c rows + k-reduce + add a^T
                        for mt in range(n_mt):
                            Fm = min(128, F - 128 * mt)
                            Fg = ((Fm + 15) // 16) * 16
                            gath = gpool.tile([128, K * 128], f32, tag="gath")
                            nc.gpsimd.ap_gather(
                                gath[0:Fg, :],
                                CTs[mt][0:Fg, :].rearrange("c (n d) -> c n d", d=1),
                                idxw[0:Fg, :],
                                channels=Fg, num_elems=P, d=1, num_idxs=K * 128)
                            dst = outs_mt[mt] if li == 3 else outs_mt[0]
                            pending.append((gath, Fm, dst, mt, tc0, tc1))

                    flush_pending()

                # ---------------- cat @ Wm + pooling ----------------
                GP = cpool.tile([128, 16], f32)
                cat_kts = [(L2, 0, 64), (L3, 0, 64), (L4, 0, 128),
                           (X4a, 0, 128), (X4b, 0, 128)]
                # round cat features into f32r tiles (ACT is idle; selection
                # is all done so rounding is safe here)
                catr = []
                for i, (buf, r0, r1) in enumerate(cat_kts):
                    cr_ = cpool.tile([r1 - r0, P], f32r, tag=f"catr{i}")
                    nc.scalar.activation(cr_[:], buf[r0:r1, :], ACTF.Copy)
                    catr.append((cr_, 0, r1 - r0))
                cat_kts = catr
                for mt in range(8):
                    mc0, mc1 = 128 * mt, 128 * (mt + 1)
                    ops = pspool.tile([128, P], f32, tag="s")
                    for n in range(2):
                        for kt, ((buf, r0, r1), wmk) in enumerate(zip(cat_kts, wm_sb)):
                            nc.tensor.matmul(
                                ops[:, 512 * n:512 * (n + 1)],
                                wmk[:, mc0:mc1],
                                buf[r0:r1, 512 * n:512 * (n + 1)],
                                start=(kt == 0), stop=(kt == 4))
                    nc.vector.tensor_reduce(GP[:, mt:mt + 1], ops[:], axis=AG.X,
                                            op=ALU.max)
                    # gmean: sum via ACT accumulate during PSUM->SBUF drain
                    osb = opool.tile([128, P], f32, tag="osb")
                    nc.scalar.activation(osb[:], ops[:], ACTF.Copy,
                                         accum_out=GP[:, 8 + mt:9 + mt])
                # gmax += bm ; gmean = gmean/P + bm
                nc.vector.tensor_tensor(GP[:, 0:8], GP[:, 0:8], bm_sb[:], op=ALU.add)
                nc.vector.tensor_scalar(GP[:, 8:16], GP[:, 8:16], 1.0 / P, None,
                                        op0=ALU.mult)
                nc.vector.tensor_tensor(GP[:, 8:16], GP[:, 8:16], bm_sb[:], op=ALU.add)

                # pooled [2048] -> cc_in, AllGather
                for m in range(16):
                    nc.sync.dma_start(cc_in[0:1, 128 * m:128 * (m + 1)],
                                      GP[:, m:m + 1])
                nc.gpsimd.collective_compute(
                    "AllGather", ALU.bypass,
                    replica_groups=[list(range(N_CORES))],
                    ins=[cc_in[:].opt()], outs=[cc_out[:].opt()])

                # ---------------- head (redundant on every core) ----------------
                HT = cpool.tile([128, 128], f32)   # H^T K-tiles: col block k = [128,8]
                for k in range(16):
                    nc.sync.dma_start(
                        HT[:, 8 * k:8 * (k + 1)],
                        cc_out[:, 128 * k:128 * (k + 1)].rearrange("b f -> f b"))

                def bn_leaky(src, blocks, gamma, beta):
                    # src [128, 8*blocks]; batch-norm over free dim (batch) then leaky
                    for mt in range(blocks):
                        blk = src[:, 8 * mt:8 * (mt + 1)]
                        mu = wpool.tile([128, 1], f32, tag="mu")
                        nc.vector.tensor_reduce(mu[:], blk, axis=AG.X, op=ALU.add)
                        nc.vector.tensor_scalar(mu[:], mu[:], 1.0 / 8, None, op0=ALU.mult)
                        nc.vector.tensor_scalar(blk, blk, mu[:], None, op0=ALU.subtract)
                        sq2 = wpool.tile([128, 8], f32, tag="sq2")
                        nc.scalar.activation(sq2[:], blk, ACTF.Square)
                        var = wpool.tile([128, 1], f32, tag="var")
                        nc.vector.tensor_reduce(var[:], sq2[:], axis=AG.X, op=ALU.add)
                        nc.scalar.activation(var[:], var[:], ACTF.Sqrt,
                                             scale=1.0 / 8, bias=EPSC[:])
                        nc.vector.reciprocal(var[:], var[:])
                        nc.vector.tensor_scalar(blk, blk, var[:], None, op0=ALU.mult)
                        nc.vector.tensor_scalar(blk, blk, gamma[:, mt:mt + 1],
                                                beta[:, mt:mt + 1], op0=ALU.mult,
                                                op1=ALU.add)
                        lk = wpool.tile([128, 8], f32, tag="lk")
                        nc.vector.tensor_scalar(lk[:], blk, 0.2, None, op0=ALU.mult)
                        nc.vector.tensor_tensor(blk, blk, lk[:], op=ALU.max)

                HA = cpool.tile([128, 32], f32)
                for mt in range(4):
                    hps = ps2pool.tile([128, 8], f32, tag="pre")
                    for k in range(16):
                        nc.tensor.matmul(hps[:], wa_sb[k][:, 128 * mt:128 * (mt + 1)],
                                         HT[:, 8 * k:8 * (k + 1)],
                                         start=(k == 0), stop=(k == 15))
                    nc.scalar.activation(HA[:, 8 * mt:8 * (mt + 1)], hps[:], ACTF.Identity,
                                         bias=ba_sb[:, mt:mt + 1])
                bn_leaky(HA, 4, ga_sb, bea_sb)

                HB = cpool.tile([128, 16], f32)
                for mt in range(2):
                    hps = ps2pool.tile([128, 8], f32, tag="pre")
                    for k in range(4):
                        nc.tensor.matmul(hps[:], wbh_sb[k][:, 128 * mt:128 * (mt + 1)],
                                         HA[:, 8 * k:8 * (k + 1)],
                                         start=(k == 0), stop=(k == 3))
                    nc.scalar.activation(HB[:, 8 * mt:8 * (mt + 1)], hps[:], ACTF.Identity,
                                         bias=bbh_sb[:, mt:mt + 1])
                bn_leaky(HB, 2, gb_sb, beb_sb)

                ops2 = ps2pool.tile([128, 8], f32, tag="pre")
                for k in range(2):
                    nc.tensor.matmul(ops2[0:40, :], wc_sb[k][:, :],
                                     HB[:, 8 * k:8 * (k + 1)],
                                     start=(k == 0), stop=(k == 1))
                outs = cpool.tile([40, 8], f32)
                nc.scalar.activation(outs[:], ops2[0:40, :], ACTF.Identity, bias=bc_sb[:])
                nc.sync.dma_start(y_out[:].rearrange("b f -> f b"), outs[:])

    nc.finalize()
    return nc


def _prep_inputs(inputs):
    """Host-side sharding + weight reparametrization; all fp32."""
    f = np.float32
    pos = np.ascontiguousarray(inputs["pos"], dtype=f).reshape(B, P, 3)
    maps = []
    names = [("W1", "b1"), ("W2", "b2"), ("W3", "b3"), ("W4", "b4")]
    common = {}
    for li, (C, F) in enumerate(LAYERS):
        W = np.asarray(inputs[names[li][0]], dtype=f)
        b = np.asarray(inputs[names[li][1]], dtype=f)
        common[f"wsub{li}"] = np.ascontiguousarray(W[:C] - W[C:])
        common[f"whalf{li}"] = np.ascontiguousarray(W[C:])
        common[f"bvec{li}"] = b.reshape(F, 1)
    common["wm"] = np.asarray(inputs["Wm"], dtype=f)
    common["bm"] = np.asarray(inputs["bm"], dtype=f).reshape(1024, 1)
    common["wa"] = np.asarray(inputs["Wa"], dtype=f)
    common["ba"] = np.asarray(inputs["ba"], dtype=f).reshape(512, 1)
    common["ga"] = np.asarray(inputs["ga"], dtype=f).reshape(512, 1)
    common["bea"] = np.asarray(inputs["bea"], dtype=f).reshape(512, 1)
    common["wbh"] = np.asarray(inputs["Wb"], dtype=f)
    common["bbh"] = np.asarray(inputs["bb"], dtype=f).reshape(256, 1)
    common["gb"] = np.asarray(inputs["gb"], dtype=f).reshape(256, 1)
    common["beb"] = np.asarray(inputs["beb"], dtype=f).reshape(256, 1)
    common["wc"] = np.asarray(inputs["Wc"], dtype=f)
    common["bc"] = np.asarray(inputs["bc"], dtype=f).reshape(40, 1)
    for c in range(N_CORES):
        m = dict(common)
        m["posT"] = np.ascontiguousarray(pos[c].T)
        maps.append(m)
    return maps


def kernel(**inputs) -> np.ndarray:
    from concourse.bass_utils import run_bass_kernel_spmd

    if "nc" not in _cache:
        _cache["nc"] = _build()
    nc = _cache["nc"]
    in_maps = _prep_inputs(inputs)
    res = run_bass_kernel_spmd(nc, in_maps, core_ids=list(range(N_CORES)))
    return np.asarray(res.results[0]["y"], dtype=np.float32)
